# revision 12
# baseline (speedup 1.0000x reference)
"""AdaptiveResidualFeatureRefinement Trainium2 kernel.

Data-parallel over the batch: B=8 samples -> 8 NeuronCores, one sample per
core, weights replicated. Per core (C=256 channels split into 2 chunks of
128 partitions):

  fine branch   : two residual dilated 3x3 convs as 18 accumulating
                  TensorE matmuls per output tile (9 taps x 2 cin chunks);
                  the "+x" residual is folded into the center conv tap
                  (w[:, :, 1, 1] += I) on the host; bias+ReLU on ScalarE.
  coarse branch : depthwise 3x3 as 9 per-partition tensor_scalar products
                  + tensor_tensor accumulates on VectorE (2-4x perf modes),
                  ReLU on VectorE, then the 1x1 pointwise conv as 2
                  accumulating matmuls per tile, bias+ReLU on ScalarE.
  routing       : per-pixel select via copy_predicated on VectorE using a
                  host-precomputed replicated uint8 mask.

All TensorE compute in bf16 (fp32 PSUM accumulate). Inputs are repacked /
padded / cast on host inside kernel().
"""

import sys
import types

import numpy as np
import ml_dtypes

BF16 = ml_dtypes.bfloat16

B, C, H, W = 8, 256, 128, 128
N_CORES = 8
KC = 2          # input-channel chunks of 128
MC = 2          # output-channel chunks of 128
HW = H * W      # 16384
PXA = 132       # x pad=2 each side (dilation-2 conv + depthwise)
PF1 = 136       # fine1 pad=4 each side (dilation-4 conv)
GROUPS = 8      # row groups of 16 rows
BLK = 4         # row blocks of 4 rows per group (N=512 matmuls)

_nc_cache = None


def _build_nc():
    import concourse.bass as bass
    import concourse.mybir as mybir
    import concourse.tile as tile
    from concourse import bacc

    DT = mybir.dt
    AF = mybir.ActivationFunctionType
    OP = mybir.AluOpType

    nc = bacc.Bacc(None, target_bir_lowering=False)

    xp_d = nc.dram_tensor("xp", [KC, 128, PXA, PXA], DT.bfloat16, kind="ExternalInput")
    w1_d = nc.dram_tensor("w1t", [128, 36, 128], DT.bfloat16, kind="ExternalInput")
    w2_d = nc.dram_tensor("w2t", [128, 36, 128], DT.bfloat16, kind="ExternalInput")
    wp_d = nc.dram_tensor("wpt", [128, 4, 128], DT.bfloat16, kind="ExternalInput")
    dw_d = nc.dram_tensor("dws", [128, 18], DT.float32, kind="ExternalInput")
    bi_d = nc.dram_tensor("bias", [128, 8], DT.float32, kind="ExternalInput")
    mk_d = nc.dram_tensor("mask", [128, HW], DT.uint8, kind="ExternalInput")
    out_d = nc.dram_tensor("out", [MC, 128, H, W], DT.float32, kind="ExternalOutput")

    with tile.TileContext(nc) as tc:
        with (
            tc.tile_pool(name="big", bufs=1) as big,
            tc.tile_pool(name="wk", bufs=4) as wk,
            tc.tile_pool(name="ps", bufs=8, space="PSUM") as ps,
        ):
            xp_t = [big.tile([128, PXA, PXA], DT.bfloat16, tag=f"xp{k}", name=f"xp{k}") for k in range(KC)]
            f1_t = [big.tile([128, PF1, PF1], DT.bfloat16, tag=f"f1{k}", name=f"f1{k}") for k in range(KC)]
            w1_t = big.tile([128, 36, 128], DT.bfloat16, tag="w1")
            w2_t = big.tile([128, 36, 128], DT.bfloat16, tag="w2")
            wp_t = big.tile([128, 4, 128], DT.bfloat16, tag="wp")
            dw_t = big.tile([128, 18], DT.float32, tag="dw")
            bi_t = big.tile([128, 8], DT.float32, tag="bi")

            nc.sync.dma_start(w1_t[:, 0:18, :], w1_d[:, 0:18, :])
            nc.sync.dma_start(bi_t[:], bi_d[:])
            nc.sync.dma_start(w1_t[:, 18:36, :], w1_d[:, 18:36, :])
            nc.sync.dma_start(dw_t[:], dw_d[:])
            for k in range(KC):
                nc.gpsimd.memset(f1_t[k][:], 0.0)
            # stage x in row slabs so conv1 group 0 starts early
            slabs = [(16 * s, 16 * s + 16) for s in range(7)] + [(112, PXA)]
            for k in range(KC):  # kc-major: the first 9 matmul rounds read only kc=0
                for r0, r1 in slabs:
                    nc.sync.dma_start(xp_t[k][:, r0:r1, :], xp_d[k, :, r0:r1, :])
            nc.sync.dma_start(w2_t[:], w2_d[:])
            nc.sync.dma_start(wp_t[:], wp_d[:])

            # bias columns: 0,1=b1  2,3=b2  4,5=bd  6,7=bp  (per cout chunk)

            # ---- Phase A: fine1 = relu(x + conv1(x) + b1) -> f1_t (padded) ----
            # 32-row groups use all 8 PSUM banks: one weight load feeds 8 matmuls
            for mc in range(MC):
                for g in range(4):
                    psl = [ps.tile([128, BLK, 128], DT.float32, tag="mm", name="mm") for _ in range(8)]
                    for kc in range(KC):
                        for t in range(9):
                            ky, kx = divmod(t, 3)
                            lhs = w1_t[:, mc * 18 + kc * 9 + t, :]
                            for b in range(8):
                                h = g * 32 + b * 4
                                r0 = h + 2 * ky  # = 2 + h + 2*(ky-1)
                                c0 = 2 * kx      # = 2 + 2*(kx-1)
                                nc.tensor.matmul(
                                    psl[b][:],
                                    lhs,
                                    xp_t[kc][:, r0:r0 + 4, c0:c0 + 128],
                                    start=(kc == 0 and t == 0),
                                    stop=(kc == 1 and t == 8),
                                )
                    for b in range(8):
                        h = g * 32 + b * 4
                        nc.scalar.activation(
                            f1_t[mc][:, 4 + h:4 + h + 4, 4:132],
                            psl[b][:],
                            AF.Relu,
                            bias=bi_t[:, mc:mc + 1],
                        )

            # ---- Phase B: depthwise+pointwise coarse, conv2 fine, select ----
            for g in range(GROUPS):
                # depthwise 3x3 (d=1) on x rows 16g..16g+15 -> coarse1 (bf16)
                c1_t = []
                for kc in range(KC):
                    ct = wk.tile([128, 16, 128], DT.bfloat16, tag="c1", name="c1", bufs=4)
                    for t in range(9):
                        ky, kx = divmod(t, 3)
                        src = xp_t[kc][:, 16 * g + ky + 1:16 * g + ky + 17, kx + 1:kx + 129]
                        sc = dw_t[:, kc * 9 + t:kc * 9 + t + 1]
                        if t == 0:
                            nc.vector.tensor_scalar(
                                ct[:], src, sc, bi_t[:, 4 + kc:5 + kc], OP.mult, OP.add
                            )
                        else:
                            # tensor_scalar (2-4x mode) + tensor_tensor add (2x)
                            # beats 1x-only scalar_tensor_tensor
                            p = wk.tile([128, 16, 128], DT.bfloat16, tag="dwp", name="dwp", bufs=2)
                            nc.vector.tensor_scalar_mul(p[:], src, sc)
                            nc.vector.tensor_tensor(ct[:], ct[:], p[:], OP.add)
                    nc.vector.tensor_scalar_max(ct[:], ct[:], 0.0)
                    c1_t.append(ct)

                # fine2 = relu(fine1 + conv2(fine1) + b2); coarse2 = relu(pw+bp)
                for mc in range(MC):
                    psl = [ps.tile([128, BLK, 128], DT.float32, tag="mm", name="mm") for _ in range(BLK)]
                    for kc in range(KC):
                        for t in range(9):
                            ky, kx = divmod(t, 3)
                            lhs = w2_t[:, mc * 18 + kc * 9 + t, :]
                            for b in range(BLK):
                                h = g * 16 + b * 4
                                r0 = h + 4 * ky  # = 4 + h + 4*(ky-1)
                                c0 = 4 * kx
                                nc.tensor.matmul(
                                    psl[b][:],
                                    lhs,
                                    f1_t[kc][:, r0:r0 + 4, c0:c0 + 128],
                                    start=(kc == 0 and t == 0),
                                    stop=(kc == 1 and t == 8),
                                )
                    f2l = []
                    for b in range(BLK):
                        f2 = wk.tile([128, 512], DT.bfloat16, tag="f2")
                        nc.scalar.activation(
                            f2[:], psl[b][:], AF.Relu, bias=bi_t[:, 2 + mc:3 + mc]
                        )
                        f2l.append(f2)
                    # pointwise batched kc-outer: one wp weight load per 4 matmuls
                    ppl = [ps.tile([128, 512], DT.float32, tag="mm", name="pw") for _ in range(BLK)]
                    for kc in range(KC):
                        for b in range(BLK):
                            nc.tensor.matmul(
                                ppl[b][:], wp_t[:, kc * 2 + mc, :],
                                c1_t[kc][:, 4 * b:4 * b + 4, :],
                                start=(kc == 0), stop=(kc == 1),
                            )
                    for b in range(BLK):
                        ost = wk.tile([128, 512], DT.float32, tag="ost")
                        nc.scalar.activation(
                            ost[:], ppl[b][:], AF.Relu, bias=bi_t[:, 6 + mc:7 + mc]
                        )
                        mt = wk.tile([128, 512], DT.uint8, tag="mk")
                        s = (16 * g + 4 * b) * 128
                        nc.sync.dma_start(mt[:], mk_d[:, s:s + 512])
                        nc.vector.copy_predicated(ost[:], mt[:], f2l[b][:])
                        nc.sync.dma_start(out_d[mc, :, 16 * g + 4 * b:16 * g + 4 * b + 4, :], ost[:])

    nc.compile()
    return nc


def _get_nc():
    global _nc_cache
    if _nc_cache is None:
        _nc_cache = _build_nc()
    return _nc_cache


def _register_profile_hook():
    """NTFF profiling hook (this image's antenv lacks axon_hooks)."""
    try:
        if "antenv.axon_hooks" in sys.modules:
            return
        try:
            from trn_agent_boot.trn_boot import _ntff_profile_via_ctypes
            hook = _ntff_profile_via_ctypes("/opt/axon/libaxon_pjrt.so")
        except Exception:
            hook = None
        mod = types.ModuleType("antenv.axon_hooks")
        mod.get_axon_ntff_profile_hook = lambda: hook
        mod.set_axon_ntff_profile_hook = lambda h: None
        sys.modules["antenv.axon_hooks"] = mod
    except Exception:
        pass


def _pack_inputs(x, complexity_map, w1, b1, w2, b2, wd, bd, wp, bp):
    """Host-side repack: pad + cast to bf16, lhsT weight layouts, mask."""
    x = np.asarray(x, np.float32)
    cm = np.asarray(complexity_map, np.float32)
    w1 = np.asarray(w1, np.float32)
    w2 = np.asarray(w2, np.float32)
    wd = np.asarray(wd, np.float32)
    wp = np.asarray(wp, np.float32)
    b1 = np.asarray(b1, np.float32)
    b2 = np.asarray(b2, np.float32)
    bd = np.asarray(bd, np.float32)
    bp = np.asarray(bp, np.float32)

    # residual "x +" folds into the center tap: conv(x, w + I at (1,1)) = x + conv(x, w)
    eye = np.eye(C, dtype=np.float32)
    w1 = w1.copy(); w1[:, :, 1, 1] += eye
    w2 = w2.copy(); w2[:, :, 1, 1] += eye

    # w1t[k, mc*18 + kc*9 + t, m] = w1[mc*128+m, kc*128+k, ky, kx]
    def conv_lhsT(w):
        r = w.reshape(MC, 128, KC, 128, 9)          # [mc, m, kc, k, t]
        r = r.transpose(3, 0, 2, 4, 1)              # [k, mc, kc, t, m]
        return np.ascontiguousarray(r.reshape(128, 36, 128).astype(BF16))

    w1t = conv_lhsT(w1)
    w2t = conv_lhsT(w2)

    # wpt[k, kc*2+mc, m] = wp[mc*128+m, kc*128+k]
    r = wp.reshape(MC, 128, KC, 128)
    wpt = np.ascontiguousarray(r.transpose(3, 2, 0, 1).reshape(128, 4, 128).astype(BF16))

    # dws[ch, kc*9+t] = wd[kc*128+ch, 0, t]
    dws = np.ascontiguousarray(
        wd.reshape(KC, 128, 9).transpose(1, 0, 2).reshape(128, 18).astype(np.float32)
    )

    bias = np.ascontiguousarray(
        np.stack(
            [
                b1[:128], b1[128:], b2[:128], b2[128:],
                bd[:128], bd[128:], bp[:128], bp[128:],
            ],
            axis=1,
        ).astype(np.float32)
    )

    per_core = []
    for s in range(B):
        xp = np.zeros((KC, 128, PXA, PXA), BF16)
        xp[:, :, 2:2 + H, 2:2 + W] = x[s].reshape(KC, 128, H, W)
        m = (cm[s, 0] > 0.5).astype(np.uint8).reshape(1, HW)
        mask = np.ascontiguousarray(np.broadcast_to(m, (128, HW)))
        per_core.append(
            {
                "xp": xp,
                "w1t": w1t,
                "w2t": w2t,
                "wpt": wpt,
                "dws": dws,
                "bias": bias,
                "mask": mask,
            }
        )
    return per_core


def kernel(x, complexity_map, w1, b1, w2, b2, wd, bd, wp, bp, _trace=False):
    _register_profile_hook()
    import concourse.bass_utils as bass_utils

    bass_utils.upload_artifacts = lambda tmpdir: f"local://{tmpdir}"

    nc = _get_nc()
    in_maps = _pack_inputs(x, complexity_map, w1, b1, w2, b2, wd, bd, wp, bp)
    last_err = None
    for _attempt in range(3):
        try:
            res = bass_utils.run_bass_kernel_spmd(
                nc, in_maps, core_ids=list(range(N_CORES)), trace=_trace
            )
            break
        except Exception as e:  # transient NRT/device faults — retry
            last_err = e
            try:  # re-init the PJRT client so a wedged device can recover
                import jax

                jax.clear_caches()
                clear = getattr(jax, "clear_backends", None) or getattr(
                    jax.extend.backend, "clear_backends", None
                )
                if clear is not None:
                    clear()
            except Exception:
                pass
    else:
        raise last_err
    out = np.stack(
        [res.results[i]["out"].reshape(C, H, W) for i in range(N_CORES)]
    ).astype(np.float32)
    if _trace:
        kernel.last_exec_time_ns = res.exec_time_ns
        kernel.last_profile = res
    return out


# revision 13
# speedup vs baseline: 1.0003x; 1.0003x over previous
"""AdaptiveResidualFeatureRefinement Trainium2 kernel.

Data-parallel over the batch: B=8 samples -> 8 NeuronCores, one sample per
core, weights replicated. Per core (C=256 channels split into 2 chunks of
128 partitions):

  fine branch   : two residual dilated 3x3 convs as 18 accumulating
                  TensorE matmuls per output tile (9 taps x 2 cin chunks);
                  the "+x" residual is folded into the center conv tap
                  (w[:, :, 1, 1] += I) on the host; bias+ReLU on ScalarE.
  coarse branch : depthwise 3x3 as 9 per-partition tensor_scalar products
                  + tensor_tensor accumulates on VectorE (2-4x perf modes),
                  ReLU on VectorE, then the 1x1 pointwise conv as 2
                  accumulating matmuls per tile, bias+ReLU on ScalarE.
  routing       : per-pixel select via copy_predicated on VectorE using a
                  host-precomputed replicated uint8 mask.

All TensorE compute in bf16 (fp32 PSUM accumulate). Inputs are repacked /
padded / cast on host inside kernel().
"""

import sys
import types

import numpy as np
import ml_dtypes

BF16 = ml_dtypes.bfloat16

B, C, H, W = 8, 256, 128, 128
N_CORES = 8
KC = 2          # input-channel chunks of 128
MC = 2          # output-channel chunks of 128
HW = H * W      # 16384
PXA = 132       # x pad=2 each side (dilation-2 conv + depthwise)
PF1 = 136       # fine1 pad=4 each side (dilation-4 conv)
GROUPS = 8      # row groups of 16 rows
BLK = 4         # row blocks of 4 rows per group (N=512 matmuls)

_nc_cache = None


def _build_nc():
    import concourse.bass as bass
    import concourse.mybir as mybir
    import concourse.tile as tile
    from concourse import bacc

    DT = mybir.dt
    AF = mybir.ActivationFunctionType
    OP = mybir.AluOpType

    nc = bacc.Bacc(None, target_bir_lowering=False)

    xp_d = nc.dram_tensor("xp", [KC, 128, PXA, PXA], DT.bfloat16, kind="ExternalInput")
    w1_d = nc.dram_tensor("w1t", [128, 36, 128], DT.bfloat16, kind="ExternalInput")
    w2_d = nc.dram_tensor("w2t", [128, 36, 128], DT.bfloat16, kind="ExternalInput")
    wp_d = nc.dram_tensor("wpt", [128, 4, 128], DT.bfloat16, kind="ExternalInput")
    dw_d = nc.dram_tensor("dws", [128, 18], DT.float32, kind="ExternalInput")
    bi_d = nc.dram_tensor("bias", [128, 8], DT.float32, kind="ExternalInput")
    mk_d = nc.dram_tensor("mask", [128, HW], DT.uint8, kind="ExternalInput")
    out_d = nc.dram_tensor("out", [MC, 128, H, W], DT.float32, kind="ExternalOutput")

    with tile.TileContext(nc) as tc:
        with (
            tc.tile_pool(name="big", bufs=1) as big,
            tc.tile_pool(name="wk", bufs=4) as wk,
            tc.tile_pool(name="ps", bufs=8, space="PSUM") as ps,
        ):
            xp_t = [big.tile([128, PXA, PXA], DT.bfloat16, tag=f"xp{k}", name=f"xp{k}") for k in range(KC)]
            f1_t = [big.tile([128, PF1, PF1], DT.bfloat16, tag=f"f1{k}", name=f"f1{k}") for k in range(KC)]
            w1_t = big.tile([128, 36, 128], DT.bfloat16, tag="w1")
            w2_t = big.tile([128, 36, 128], DT.bfloat16, tag="w2")
            wp_t = big.tile([128, 4, 128], DT.bfloat16, tag="wp")
            dw_t = big.tile([128, 18], DT.float32, tag="dw")
            bi_t = big.tile([128, 8], DT.float32, tag="bi")

            nc.sync.dma_start(w1_t[:, 0:18, :], w1_d[:, 0:18, :])
            nc.sync.dma_start(bi_t[:], bi_d[:])
            nc.sync.dma_start(w1_t[:, 18:36, :], w1_d[:, 18:36, :])
            nc.sync.dma_start(dw_t[:], dw_d[:])
            for k in range(KC):
                nc.gpsimd.memset(f1_t[k][:], 0.0)
            # stage x in row slabs so conv1 group 0 starts early
            slabs = [(16 * s, 16 * s + 16) for s in range(7)] + [(112, PXA)]
            for k in range(KC):  # kc-major: the first 9 matmul rounds read only kc=0
                for r0, r1 in slabs:
                    nc.sync.dma_start(xp_t[k][:, r0:r1, :], xp_d[k, :, r0:r1, :])
            nc.sync.dma_start(w2_t[:], w2_d[:])
            nc.sync.dma_start(wp_t[:], wp_d[:])

            # bias columns: 0,1=b1  2,3=b2  4,5=bd  6,7=bp  (per cout chunk)

            # ---- Phase A: fine1 = relu(x + conv1(x) + b1) -> f1_t (padded) ----
            # 32-row groups use all 8 PSUM banks: one weight load feeds 8 matmuls
            for mc in range(MC):
                for g in range(4):
                    psl = [ps.tile([128, BLK, 128], DT.float32, tag="mm", name="mm") for _ in range(8)]
                    for kc in range(KC):
                        for t in range(9):
                            ky, kx = divmod(t, 3)
                            lhs = w1_t[:, mc * 18 + kc * 9 + t, :]
                            for b in range(8):
                                h = g * 32 + b * 4
                                r0 = h + 2 * ky  # = 2 + h + 2*(ky-1)
                                c0 = 2 * kx      # = 2 + 2*(kx-1)
                                nc.tensor.matmul(
                                    psl[b][:],
                                    lhs,
                                    xp_t[kc][:, r0:r0 + 4, c0:c0 + 128],
                                    start=(kc == 0 and t == 0),
                                    stop=(kc == 1 and t == 8),
                                )
                    for b in range(8):
                        h = g * 32 + b * 4
                        nc.scalar.activation(
                            f1_t[mc][:, 4 + h:4 + h + 4, 4:132],
                            psl[b][:],
                            AF.Relu,
                            bias=bi_t[:, mc:mc + 1],
                        )

            # ---- Phase B: depthwise+pointwise coarse, conv2 fine, select ----
            for g in range(GROUPS):
                # depthwise 3x3 (d=1) on x rows 16g..16g+15 -> coarse1 (bf16)
                c1_t = []
                for kc in range(KC):
                    ct = wk.tile([128, 16, 128], DT.bfloat16, tag="c1", name="c1", bufs=4)
                    for t in range(9):
                        ky, kx = divmod(t, 3)
                        src = xp_t[kc][:, 16 * g + ky + 1:16 * g + ky + 17, kx + 1:kx + 129]
                        sc = dw_t[:, kc * 9 + t:kc * 9 + t + 1]
                        if t == 0:
                            nc.vector.tensor_scalar(
                                ct[:], src, sc, bi_t[:, 4 + kc:5 + kc], OP.mult, OP.add
                            )
                        else:
                            # tensor_scalar (2-4x mode) + tensor_tensor add (2x)
                            # beats 1x-only scalar_tensor_tensor
                            p = wk.tile([128, 16, 128], DT.bfloat16, tag="dwp", name="dwp", bufs=2)
                            nc.vector.tensor_scalar_mul(p[:], src, sc)
                            nc.vector.tensor_tensor(ct[:], ct[:], p[:], OP.add)
                    nc.vector.tensor_scalar_max(ct[:], ct[:], 0.0)
                    c1_t.append(ct)

                # fine2 = relu(fine1 + conv2(fine1) + b2); coarse2 = relu(pw+bp)
                for mc in range(MC):
                    psl = [ps.tile([128, BLK, 128], DT.float32, tag="mm", name="mm") for _ in range(BLK)]
                    for kc in range(KC):
                        for t in range(9):
                            ky, kx = divmod(t, 3)
                            lhs = w2_t[:, mc * 18 + kc * 9 + t, :]
                            for b in range(BLK):
                                h = g * 16 + b * 4
                                r0 = h + 4 * ky  # = 4 + h + 4*(ky-1)
                                c0 = 4 * kx
                                nc.tensor.matmul(
                                    psl[b][:],
                                    lhs,
                                    f1_t[kc][:, r0:r0 + 4, c0:c0 + 128],
                                    start=(kc == 0 and t == 0),
                                    stop=(kc == 1 and t == 8),
                                )
                    for b in range(BLK):
                        h = g * 16 + b * 4
                        f2 = wk.tile([128, 512], DT.bfloat16, tag="f2")
                        nc.scalar.activation(
                            f2[:], psl[b][:], AF.Relu, bias=bi_t[:, 2 + mc:3 + mc]
                        )
                        pp = ps.tile([128, 512], DT.float32, tag="mm", name="pw")
                        nc.tensor.matmul(
                            pp[:], wp_t[:, mc, :], c1_t[0][:, 4 * b:4 * b + 4, :],
                            start=True, stop=False,
                        )
                        nc.tensor.matmul(
                            pp[:], wp_t[:, 2 + mc, :], c1_t[1][:, 4 * b:4 * b + 4, :],
                            start=False, stop=True,
                        )
                        ost = wk.tile([128, 512], DT.float32, tag="ost")
                        nc.scalar.activation(
                            ost[:], pp[:], AF.Relu, bias=bi_t[:, 6 + mc:7 + mc]
                        )
                        mt = wk.tile([128, 512], DT.uint8, tag="mk")
                        s = (16 * g + 4 * b) * 128
                        nc.sync.dma_start(mt[:], mk_d[:, s:s + 512])
                        nc.vector.copy_predicated(ost[:], mt[:], f2[:])
                        nc.sync.dma_start(out_d[mc, :, 16 * g + 4 * b:16 * g + 4 * b + 4, :], ost[:])

    nc.compile()
    return nc


def _get_nc():
    global _nc_cache
    if _nc_cache is None:
        _nc_cache = _build_nc()
    return _nc_cache


def _register_profile_hook():
    """NTFF profiling hook (this image's antenv lacks axon_hooks)."""
    try:
        if "antenv.axon_hooks" in sys.modules:
            return
        try:
            from trn_agent_boot.trn_boot import _ntff_profile_via_ctypes
            hook = _ntff_profile_via_ctypes("/opt/axon/libaxon_pjrt.so")
        except Exception:
            hook = None
        mod = types.ModuleType("antenv.axon_hooks")
        mod.get_axon_ntff_profile_hook = lambda: hook
        mod.set_axon_ntff_profile_hook = lambda h: None
        sys.modules["antenv.axon_hooks"] = mod
    except Exception:
        pass


def _pack_inputs(x, complexity_map, w1, b1, w2, b2, wd, bd, wp, bp):
    """Host-side repack: pad + cast to bf16, lhsT weight layouts, mask."""
    x = np.asarray(x, np.float32)
    cm = np.asarray(complexity_map, np.float32)
    w1 = np.asarray(w1, np.float32)
    w2 = np.asarray(w2, np.float32)
    wd = np.asarray(wd, np.float32)
    wp = np.asarray(wp, np.float32)
    b1 = np.asarray(b1, np.float32)
    b2 = np.asarray(b2, np.float32)
    bd = np.asarray(bd, np.float32)
    bp = np.asarray(bp, np.float32)

    # residual "x +" folds into the center tap: conv(x, w + I at (1,1)) = x + conv(x, w)
    eye = np.eye(C, dtype=np.float32)
    w1 = w1.copy(); w1[:, :, 1, 1] += eye
    w2 = w2.copy(); w2[:, :, 1, 1] += eye

    # w1t[k, mc*18 + kc*9 + t, m] = w1[mc*128+m, kc*128+k, ky, kx]
    def conv_lhsT(w):
        r = w.reshape(MC, 128, KC, 128, 9)          # [mc, m, kc, k, t]
        r = r.transpose(3, 0, 2, 4, 1)              # [k, mc, kc, t, m]
        return np.ascontiguousarray(r.reshape(128, 36, 128).astype(BF16))

    w1t = conv_lhsT(w1)
    w2t = conv_lhsT(w2)

    # wpt[k, kc*2+mc, m] = wp[mc*128+m, kc*128+k]
    r = wp.reshape(MC, 128, KC, 128)
    wpt = np.ascontiguousarray(r.transpose(3, 2, 0, 1).reshape(128, 4, 128).astype(BF16))

    # dws[ch, kc*9+t] = wd[kc*128+ch, 0, t]
    dws = np.ascontiguousarray(
        wd.reshape(KC, 128, 9).transpose(1, 0, 2).reshape(128, 18).astype(np.float32)
    )

    bias = np.ascontiguousarray(
        np.stack(
            [
                b1[:128], b1[128:], b2[:128], b2[128:],
                bd[:128], bd[128:], bp[:128], bp[128:],
            ],
            axis=1,
        ).astype(np.float32)
    )

    per_core = []
    for s in range(B):
        xp = np.zeros((KC, 128, PXA, PXA), BF16)
        xp[:, :, 2:2 + H, 2:2 + W] = x[s].reshape(KC, 128, H, W)
        m = (cm[s, 0] > 0.5).astype(np.uint8).reshape(1, HW)
        mask = np.ascontiguousarray(np.broadcast_to(m, (128, HW)))
        per_core.append(
            {
                "xp": xp,
                "w1t": w1t,
                "w2t": w2t,
                "wpt": wpt,
                "dws": dws,
                "bias": bias,
                "mask": mask,
            }
        )
    return per_core


def kernel(x, complexity_map, w1, b1, w2, b2, wd, bd, wp, bp, _trace=False):
    _register_profile_hook()
    import concourse.bass_utils as bass_utils

    bass_utils.upload_artifacts = lambda tmpdir: f"local://{tmpdir}"

    nc = _get_nc()
    in_maps = _pack_inputs(x, complexity_map, w1, b1, w2, b2, wd, bd, wp, bp)
    last_err = None
    for _attempt in range(3):
        try:
            res = bass_utils.run_bass_kernel_spmd(
                nc, in_maps, core_ids=list(range(N_CORES)), trace=_trace
            )
            break
        except Exception as e:  # transient NRT/device faults — retry
            last_err = e
            try:  # re-init the PJRT client so a wedged device can recover
                import jax

                jax.clear_caches()
                clear = getattr(jax, "clear_backends", None) or getattr(
                    jax.extend.backend, "clear_backends", None
                )
                if clear is not None:
                    clear()
            except Exception:
                pass
    else:
        raise last_err
    out = np.stack(
        [res.results[i]["out"].reshape(C, H, W) for i in range(N_CORES)]
    ).astype(np.float32)
    if _trace:
        kernel.last_exec_time_ns = res.exec_time_ns
        kernel.last_profile = res
    return out
